# revision 6
# baseline (speedup 1.0000x reference)
"""Trainium2 Bass kernel for nn_LinearAttention (B=8, C=256, H=W=64, 4 heads x 128).

Strategy
--------
Data-parallel over batch: each of the 8 NeuronCores processes one batch
element end-to-end (no collectives).

Per-core math (x is [C=256, n=4096], weights from the 1x1 convs):
    k^T = x^T @ w_k^T            [n, 512]  (n on partitions)
    e   = exp(k^T)               (softmax without max-subtraction; |k| <~ 5)
    G_h = e_h^T @ [x^T | 1]      [128, 257] accumulated over n-tiles on PSUM;
                                 col 256 gives the softmax row-sums for free.
    G~  = G / rowsum             (per-partition scale during the PSUM drain)
    P_h = G~_h^T @ w_q_h         [256, 256]  } the "algebraic collapse":
    W^T = sum_h P_h @ U_h        [256, 256]  } U_h = W_v,h^T w_out_h^T is a
    out = W @ x + b              [256, 4096] } host-precomputed weight product

Schedule notes (v3):
  - tiles processed in pairs: one exp activation per [128, 1024] PSUM
    2-bank region halves the Scalar-engine instruction count.
  - ONE persistent PSUM layout (8 banks): G0-G3 on banks 0-3, two
    [128,1024] k^T pair buffers on banks 4-7.  The collapse and final
    phases write into manually aliased slices of those same banks, so
    no pool open/close barrier serializes the join (a fresh pool would
    wait on the LAST drain of the previous pool's banks).  The PE
    program order P0 P1 W0 P2 W1 P3 W2 W3 makes every whole-bank
    zeroing (matmul start=True) transitively ordered after the copies
    that still read the bank.
  - warm-up matmuls sized to the measured ~12us first-chunk-ready time
    (first-byte floor ~8.7us + ~3us stream under 8-core contention);
    any PE idle gap >~2us re-throttles the HAM clock gate to 1.2 GHz.
  - PSUM drains are full-width (per-instruction overhead ~230ns
    dominates half-width drains) and alternate Scalar/Vector.
  - the last output chunk's DMA is split per 128-row block so the final
    HBM write (and its completion receipt) covers only 128 KB.

All matmuls use bf16 operands with fp32 PSUM accumulation. Inputs are
packed host-side into one bf16 stream ordered exactly in consumption
order (wk first, then per-tile [x-tile | x^T-tile | ones]) plus a bf16
weight stream (wq | U). The kernel output is bf16 [128, 2, 4096]
(host re-assembles).
"""

import numpy as np

HEADS = 4
DH = 128
C = 256
HID = 512
N = 4096
NT = N // 128  # 32 n-tiles
NCORES = 8

TILE_COLS = 513           # per-tile bf16 cols: 256 (x k-blocks) + 257 (xT|1)
XB = 2 * HID              # 1024 cols of wk at the head of the bf16 stream
XALL_COLS = XB + NT * TILE_COLS   # 17440
WF_COLS = 3 * 1024        # wq | U (bf16)

_BUILD_CACHE = {}


def _build_program():
    """Build + compile the SPMD Bass program (same NEFF for all 8 cores)."""
    from contextlib import ExitStack

    import concourse.bass as bass
    import concourse.tile as tile
    from concourse import bacc, mybir

    f32 = mybir.dt.float32
    bf16 = mybir.dt.bfloat16
    AFT = mybir.ActivationFunctionType

    nc = bacc.Bacc(
        "TRN2", target_bir_lowering=False, debug=False, num_devices=1
    )

    xall_d = nc.dram_tensor("xall", [128, XALL_COLS], bf16, kind="ExternalInput").ap()
    wf_d = nc.dram_tensor("wf", [128, WF_COLS], bf16, kind="ExternalInput").ap()
    bb_d = nc.dram_tensor("bb", [128, 2], f32, kind="ExternalInput").ap()
    out_d = nc.dram_tensor("out", [128, 2 * N], bf16, kind="ExternalOutput").ap()

    with tile.TileContext(nc) as tc, ExitStack() as stack:
        const = stack.enter_context(tc.tile_pool(name="const", bufs=1))
        psum = stack.enter_context(tc.tile_pool(name="psum", bufs=1, space="PSUM"))

        xall_sb = const.tile([128, XALL_COLS], bf16)
        wf_sb = const.tile([128, WF_COLS], bf16)
        bb_sb = const.tile([128, 2], f32)
        # zero tiles for PE warm-up matmuls (no DMA dependency); a small
        # one on GpSimd unblocks N=128 warms ~600ns before the Vector
        # memset of the N=512 operand completes.
        zts = const.tile([128, 128], bf16)
        zt = const.tile([128, 512], bf16)
        nc.gpsimd.memset(zts[:], 0.0)
        nc.vector.memset(zt[:], 0.0)

        def col(i):  # first col of tile i's block
            return XB + i * TILE_COLS

        # DMA-descriptor issue on Sync costs ~0.65us per dma_start; chunk
        # sizes grow so each chunk's completion sem fires just before its
        # first tile-pair is consumed (pair cadence ~1.75us, measured
        # stream rate ~0.26 GB/ms under 8-core contention).
        nc.sync.dma_start(xall_sb[:, 0 : col(2)], xall_d[:, 0 : col(2)])
        nc.sync.dma_start(xall_sb[:, col(2) : col(4)], xall_d[:, col(2) : col(4)])
        nc.sync.dma_start(xall_sb[:, col(4) : col(8)], xall_d[:, col(4) : col(8)])
        nc.sync.dma_start(xall_sb[:, col(8) : col(16)], xall_d[:, col(8) : col(16)])
        nc.sync.dma_start(xall_sb[:, col(16) : col(24)], xall_d[:, col(16) : col(24)])
        nc.sync.dma_start(xall_sb[:, col(24) :], xall_d[:, col(24) :])
        nc.sync.dma_start(wf_sb[:], wf_d[:])
        nc.sync.dma_start(bb_sb[:], bb_d[:])

        def wk(k):  # rhs: w_k^T block for C-rows k*128..+128 -> [128, 512]
            return xall_sb[:, k * HID : (k + 1) * HID]

        def xs(k, i):  # lhsT: x rows k-block, spatial tile i -> [128, 128]
            return xall_sb[:, col(i) + k * 128 : col(i) + (k + 1) * 128]

        def xt(i):  # rhs: [x^T | 1] for spatial tile i -> [128, 257]
            return xall_sb[:, col(i) + 256 : col(i) + TILE_COLS]

        # x-tile columns as [128, tile, col] for the final streamed matmul
        x_tiles = xall_sb[:, XB:].rearrange("p (i t) -> p i t", t=TILE_COLS)

        def wq_h(h):
            return wf_sb[:, h * C : (h + 1) * C]

        def u_h(h, cb):  # U_h[c'-blk] = (W_v,h^T w_out_h^T) rows c'-blk
            o = 1024 + (2 * h + cb) * C
            return wf_sb[:, o : o + C]

        rsum = const.tile([128, HEADS], f32)
        gn_sb = const.tile([128, HEADS * C], bf16)
        p_sb = const.tile([128, HEADS * 2 * C], bf16)
        w_sb = const.tile([128, 2 * C], bf16)

        # ---- persistent PSUM layout (8 banks) ----
        # g[h] owns bank h ([:, 0:257] holds G_h; the full [:, 0:512]
        # is reused as a final-phase matmul target).  pk2[0/1] own banks
        # 4-5 / 6-7; their slices are reused for P and W^T in the join.
        g = [psum.tile([128, 512], f32, name=f"G{h}") for h in range(HEADS)]
        pk2 = [psum.tile([128, 1024], f32, name=f"pk2{a}") for a in range(2)]

        # ---- warm-ups: keep the PE busy (HAM at 8/8) until data lands ----
        for _ in range(5):
            nc.tensor.matmul(pk2[1][:, 0:128], zts[:], zts[:])
        for _ in range(10):
            nc.tensor.matmul(pk2[1][:, 0:HID], zts[:], zt[:])
        for _ in range(3):
            nc.tensor.matmul(pk2[1][:, 0:128], zts[:], zts[:])

        # ---- Phase 1: k^T projection + exp + G accumulation (tile pairs) ----
        NP = NT // 2  # 16 pairs
        with tc.tile_pool(name="ekp", bufs=3) as ekp:

            def emit_g(ek2, j, t):
                i = 2 * j + t
                for h in range(HEADS):
                    nc.tensor.matmul(
                        g[h][:, 0:257],
                        ek2[:, t * HID + h * 128 : t * HID + (h + 1) * 128],
                        xt(i),
                        start=(i == 0),
                        stop=(i == NT - 1),
                        skip_group_check=True,
                    )

            pending = []
            for j in range(NP):
                pkt = pk2[j % 2]
                for t in range(2):
                    i = 2 * j + t
                    for k in range(2):
                        nc.tensor.matmul(
                            pkt[:, t * HID : (t + 1) * HID],
                            xs(k, i),
                            wk(k),
                            start=(k == 0),
                            stop=(k == 1),
                        )
                ek2 = ekp.tile([128, 1024], bf16, name="ek2")
                if j == NP - 1:
                    # split the last exps per tile so the trailing G matmuls
                    # of tile 30 can start while tile 31's exp runs
                    nc.scalar.activation(ek2[:, 0:HID], pkt[:, 0:HID], AFT.Exp)
                    nc.scalar.activation(
                        ek2[:, HID:1024], pkt[:, HID:1024], AFT.Exp
                    )
                else:
                    nc.scalar.activation(ek2[:], pkt[:], AFT.Exp)
                # software-pipeline the G matmuls one pair behind so the
                # tensor engine never stalls on the exp of the same pair
                pending.append((ek2, j))
                if len(pending) > 1:
                    e, jj = pending.pop(0)
                    emit_g(e, jj, 0)
                    emit_g(e, jj, 1)
            e, jj = pending.pop(0)
            emit_g(e, jj, 0)
            emit_g(e, jj, 1)

            # rowsum reciprocals (DVE for accuracy), then drain+normalize
            # G into SBUF bf16, full-width, alternating Scalar/Vector.
            for h in range(HEADS):
                nc.vector.reciprocal(rsum[:, h : h + 1], g[h][:, 256:257])
            for h in range(HEADS):
                dst = gn_sb[:, h * C : (h + 1) * C]
                if h % 2 == 0:
                    nc.scalar.mul(dst, g[h][:, 0:C], rsum[:, h : h + 1])
                else:
                    nc.vector.tensor_scalar_mul(
                        dst, g[h][:, 0:C], rsum[:, h : h + 1]
                    )

            # ---- Phase 2a: collapse weights (P -> W^T), PSUM-aliased ----
            # Each head's P pair lives in its OWN bank (pk2[0]'s two banks
            # for h0/h1, the already-drained g0/g1 banks for h2/h3) so all
            # 8 P matmuls stream back-to-back with no copy in the chain;
            # the cb=1 matmul lands in the same bank with start=False (the
            # bank was zeroed by cb=0's start=True).  wt[cb] live in
            # pk2[1]'s two banks.
            wt = [pk2[1][:, cb * HID : cb * HID + C] for cb in range(2)]
            p_banks = [
                pk2[0][:, 0:HID],
                pk2[0][:, HID:1024],
                g[0][:, 0:HID],
                g[1][:, 0:HID],
            ]
            p_tiles = {}

            # two bridging warm-ups: keep the PE from re-throttling during
            # the recip+drain window (they write wt's banks, which the
            # first W matmul's start=True re-zeroes anyway)
            for _ in range(2):
                nc.tensor.matmul(pk2[1][:, 0:HID], zts[:], zt[:])

            def emit_p(h):
                for cb in range(2):
                    pt = p_banks[h][:, cb * C : (cb + 1) * C]
                    nc.tensor.matmul(
                        pt,
                        gn_sb[:, h * C + cb * 128 : h * C + (cb + 1) * 128],
                        wq_h(h),
                        start=(cb == 0),
                        stop=True,
                        skip_group_check=True,
                    )
                    p_tiles[(h, cb)] = pt

            def copy_p(h):
                for cb in range(2):
                    dst = p_sb[:, h * 2 * C + cb * C : h * 2 * C + (cb + 1) * C]
                    if cb == 0:
                        nc.scalar.copy(dst, p_tiles[(h, cb)])
                    else:
                        nc.vector.tensor_copy(dst, p_tiles[(h, cb)])

            def emit_w(h):
                # W^T[c-blk, o] += sum_{c'-blk} P_h[c'-blk, c-blk]^T @ U_h
                for pb in range(2):
                    for cb in range(2):
                        nc.tensor.matmul(
                            wt[cb],
                            p_sb[:, h * 2 * C + pb * C + cb * 128 : h * 2 * C + pb * C + cb * 128 + 128],
                            u_h(h, pb),
                            start=(h == 0 and pb == 0),
                            stop=(h == HEADS - 1 and pb == 1),
                            skip_group_check=True,
                        )

            for h in range(HEADS):
                emit_p(h)
            for h in range(HEADS):
                copy_p(h)
            for h in range(HEADS):
                emit_w(h)

            nc.scalar.copy(w_sb[:, 0:C], wt[0])
            nc.vector.tensor_copy(w_sb[:, C : 2 * C], wt[1])

        # ---- Phase 2b: out = W @ x + b, streamed over 8 chunks of 512
        # columns; PSUM targets alias the (drained) G banks.  One output
        # DMA per chunk covers both 128-row blocks, except the last chunk
        # which is split per row-block so the final HBM write is 128 KB.
        with tc.tile_pool(name="fop", bufs=4) as fop:
            for c in range(8):
                fo = fop.tile([128, 1024], bf16, name="fo")
                for mo in range(2):
                    # cycle the G banks starting at g2 (drained earliest;
                    # g0/g1 hold P(2)/P(3) until their copies complete)
                    fp_ = g[(2 + 2 * c + mo) % 4][:, 0:512]
                    for cb in range(2):
                        nc.tensor.matmul(
                            fp_,
                            w_sb[:, cb * C + mo * 128 : cb * C + mo * 128 + 128],
                            x_tiles[:, 4 * c : 4 * c + 4, cb * 128 : (cb + 1) * 128],
                            start=(cb == 0),
                            stop=(cb == 1),
                        )
                    half = fo[:, mo * 512 : (mo + 1) * 512]
                    if c == 7:
                        # last chunk: split each bias-drain across BOTH
                        # engines so the final DMAs are gated on ~350ns of
                        # drain instead of ~750ns
                        nc.scalar.activation(
                            half[:, 0:256],
                            fp_[:, 0:256],
                            AFT.Identity,
                            bias=bb_sb[:, mo : mo + 1],
                        )
                        nc.vector.tensor_scalar_add(
                            half[:, 256:512], fp_[:, 256:512], bb_sb[:, mo : mo + 1]
                        )
                        nc.sync.dma_start(
                            out_d[:, c * 1024 + mo * 512 : c * 1024 + (mo + 1) * 512],
                            half,
                        )
                    elif mo == 0:
                        nc.scalar.activation(
                            half, fp_, AFT.Identity, bias=bb_sb[:, 0:1]
                        )
                    else:
                        nc.vector.tensor_scalar_add(half, fp_, bb_sb[:, 1:2])
                # chunk-major output layout: one contiguous 2 KiB row per
                # partition per chunk (half the DMA descriptors vs split
                # row-blocks); the host unshuffles [p, c, mo, j].
                if c < 7:
                    nc.sync.dma_start(out_d[:, c * 1024 : (c + 1) * 1024], fo[:])

    nc.compile()
    return nc


def _get_program():
    if "nc" not in _BUILD_CACHE:
        _BUILD_CACHE["nc"] = _build_program()
    return _BUILD_CACHE["nc"]


def _pack_weights(w_qkv, w_out, b_out):
    """Shared (per-core-identical) input tensors: wk header cols of the bf16
    stream, the bf16 weight stream (wq | U), and the bias pair."""
    import ml_dtypes

    bf16 = ml_dtypes.bfloat16
    w_q = np.ascontiguousarray(w_qkv[0:HID]).astype(np.float32)  # [512, 256]
    w_k = np.ascontiguousarray(w_qkv[HID : 2 * HID]).astype(np.float32)
    w_v = np.ascontiguousarray(w_qkv[2 * HID : 3 * HID]).astype(np.float32)

    def pack_rows(w):  # w [512, 256] -> [128, 4*256], block h = rows h*128:+128
        return w.reshape(HEADS, 128, C).transpose(1, 0, 2).reshape(128, HEADS * C)

    wk = np.ascontiguousarray(
        w_k.T.reshape(2, 128, HID).transpose(1, 0, 2).reshape(128, 2 * HID)
    ).astype(bf16)

    # U_h = W_v,h^T @ w_out[:, h-block]^T  [256 c', 256 o]; block (h, cb) on
    # partitions = c' within block cb
    w_outf = np.ascontiguousarray(w_out).astype(np.float32)
    u_blocks = []
    for h in range(HEADS):
        u = w_v[h * 128 : (h + 1) * 128].T @ w_outf[:, h * 128 : (h + 1) * 128].T
        u_blocks.append(u.reshape(2, 128, C).transpose(1, 0, 2).reshape(128, 2 * C))
    wf = np.concatenate([pack_rows(w_q)] + u_blocks, axis=1)
    return {
        "wk": wk,  # [128, 1024] bf16 header of the xall stream
        "wf": np.ascontiguousarray(wf.astype(bf16)),
        "bb": np.ascontiguousarray(b_out.reshape(2, 128).T).astype(np.float32),
    }


def _pack_x(xb, wk):
    """Per-batch bf16 stream: [wk | per tile i: x k-blocks (256) | x^T|1 (257)].

    xb is [256, 4096] float32."""
    import ml_dtypes

    bf16 = ml_dtypes.bfloat16
    xbh = xb.astype(bf16)
    # x-tile part: [p, i, k, j] = x[k*128+p, i*128+j]
    xs = xbh.reshape(2, 128, NT, 128).transpose(1, 2, 0, 3).reshape(128, NT, 256)
    # xT part: [p, i, c] = x[c, i*128+p]
    xt = xbh.reshape(256, NT, 128).transpose(2, 1, 0)
    ones = np.ones((128, NT, 1), dtype=bf16)
    tiles = np.concatenate([xs, xt, ones], axis=2).reshape(128, NT * TILE_COLS)
    return np.ascontiguousarray(np.concatenate([wk, tiles], axis=1))


def _ensure_ntff_hook():
    """Make trace-mode grading (BASS_TRACE=1) work even when the container's
    ``antenv`` stub lacks ``axon_hooks``: install the registry module and, if
    the axon PJRT library is present, register the ctypes NTFF profile hook."""
    import os
    import sys
    import types

    try:
        import antenv.axon_hooks  # noqa: F401
    except ImportError:
        try:
            import antenv
        except ImportError:
            return
        mod = types.ModuleType("antenv.axon_hooks")
        mod._hook = None
        mod.set_axon_ntff_profile_hook = lambda h: setattr(mod, "_hook", h)
        mod.get_axon_ntff_profile_hook = lambda: getattr(mod, "_hook", None)
        sys.modules["antenv.axon_hooks"] = mod
        antenv.axon_hooks = mod
    try:
        from antenv.axon_hooks import (
            get_axon_ntff_profile_hook,
            set_axon_ntff_profile_hook,
        )

        so = "/opt/axon/libaxon_pjrt.so"
        if get_axon_ntff_profile_hook() is None and os.path.exists(so):
            from trn_agent_boot.trn_boot import _ntff_profile_via_ctypes

            hook = _ntff_profile_via_ctypes(so)
            if hook is not None:
                set_axon_ntff_profile_hook(hook)
    except Exception:
        pass


def _make_in_maps(x, w_qkv, w_out, b_out):
    packed = _pack_weights(
        np.asarray(w_qkv, np.float32),
        np.asarray(w_out, np.float32),
        np.asarray(b_out, np.float32),
    )
    wk = packed.pop("wk")
    x = np.asarray(x, dtype=np.float32)
    return [
        {"xall": _pack_x(x[b].reshape(C, N), wk), **packed}
        for b in range(x.shape[0])
    ]


def kernel(x, w_qkv, w_out, b_out):
    from concourse.bass_utils import run_bass_kernel_spmd

    _ensure_ntff_hook()

    x = np.asarray(x, dtype=np.float32)
    B = x.shape[0]
    assert B == NCORES and x.shape[1:] == (C, 64, 64)

    nc = _get_program()
    in_maps = _make_in_maps(x, w_qkv, w_out, b_out)
    res = run_bass_kernel_spmd(nc, in_maps, core_ids=list(range(NCORES)))
    # out is chunk-major: raw[p, c*1024 + mo*512 + j] = out[mo*128+p, c*512+j]
    out = np.stack(
        [
            np.asarray(res.results[b]["out"], dtype=np.float32)
            .reshape(128, 8, 2, 512)
            .transpose(2, 0, 1, 3)
            for b in range(B)
        ],
        axis=0,
    )
    return out.reshape(B, C, 64, 64).astype(np.float32)


# revision 12
# speedup vs baseline: 1.1557x; 1.1557x over previous
"""Trainium2 Bass kernel for nn_LinearAttention (B=8, C=256, H=W=64, 4 heads x 128).

Strategy
--------
Data-parallel over batch: each of the 8 NeuronCores processes one batch
element end-to-end (no collectives).

Per-core math (x is [C=256, n=4096], weights from the 1x1 convs):
    k^T = x^T @ w_k^T            [n, 512]  (n on partitions)
    e   = exp(k^T)               (softmax without max-subtraction; |k| <~ 5)
    G_h = e_h^T @ [x^T | 1]      [128, 257] accumulated over n-tiles on PSUM;
                                 col 256 gives the softmax row-sums for free.
    G~  = G / rowsum             (per-partition scale during the PSUM drain)
    P_h = G~_h^T @ w_q_h         [256, 256]  } the "algebraic collapse":
    W^T = sum_h P_h @ U_h        [256, 256]  } U_h = W_v,h^T w_out_h^T is a
    out = W @ x + b              [256, 4096] } host-precomputed weight product

Schedule notes (v3):
  - tiles processed in pairs: one exp activation per [128, 1024] PSUM
    2-bank region halves the Scalar-engine instruction count.
  - ONE persistent PSUM layout (8 banks): G0-G3 on banks 0-3, two
    [128,1024] k^T pair buffers on banks 4-7.  The collapse and final
    phases write into manually aliased slices of those same banks, so
    no pool open/close barrier serializes the join (a fresh pool would
    wait on the LAST drain of the previous pool's banks).  The PE
    program order P0 P1 W0 P2 W1 P3 W2 W3 makes every whole-bank
    zeroing (matmul start=True) transitively ordered after the copies
    that still read the bank.
  - warm-up matmuls sized to the measured ~12us first-chunk-ready time
    (first-byte floor ~8.7us + ~3us stream under 8-core contention);
    any PE idle gap >~2us re-throttles the HAM clock gate to 1.2 GHz.
  - PSUM drains are full-width (per-instruction overhead ~230ns
    dominates half-width drains) and alternate Scalar/Vector.
  - the last output chunk's DMA is split per 128-row block so the final
    HBM write (and its completion receipt) covers only 128 KB.

All matmuls use bf16 operands with fp32 PSUM accumulation. Inputs are
packed host-side into one bf16 stream ordered exactly in consumption
order (wk first, then per-tile [x-tile | x^T-tile | ones]) plus a bf16
weight stream (wq | U). The kernel output is bf16 [128, 2, 4096]
(host re-assembles).
"""

import numpy as np

HEADS = 4
DH = 128
C = 256
HID = 512
N = 4096
NT = N // 128  # 32 n-tiles
NCORES = 8

TILE_COLS = 513           # per-tile bf16 cols: 256 (x k-blocks) + 257 (xT|1)
XB = 2 * HID              # 1024 cols of wk at the head of the bf16 stream
XALL_COLS = XB + NT * TILE_COLS   # 17440
WF_COLS = 3 * 1024        # wq | U (bf16)

_BUILD_CACHE = {}


def _build_program():
    """Build + compile the SPMD Bass program (same NEFF for all 8 cores)."""
    from contextlib import ExitStack

    import concourse.bass as bass
    import concourse.tile as tile
    from concourse import bacc, mybir

    f32 = mybir.dt.float32
    bf16 = mybir.dt.bfloat16
    AFT = mybir.ActivationFunctionType

    nc = bacc.Bacc(
        "TRN2", target_bir_lowering=False, debug=False, num_devices=1
    )

    xall_d = nc.dram_tensor("xall", [128, XALL_COLS], bf16, kind="ExternalInput").ap()
    wf_d = nc.dram_tensor("wf", [128, WF_COLS], bf16, kind="ExternalInput").ap()
    bb_d = nc.dram_tensor("bb", [128, 2], f32, kind="ExternalInput").ap()
    out_d = nc.dram_tensor("out", [128, 2 * N], bf16, kind="ExternalOutput").ap()

    with tile.TileContext(nc) as tc, ExitStack() as stack:
        const = stack.enter_context(tc.tile_pool(name="const", bufs=1))
        psum = stack.enter_context(tc.tile_pool(name="psum", bufs=1, space="PSUM"))

        xall_sb = const.tile([128, XALL_COLS], bf16)
        wf_sb = const.tile([128, WF_COLS], bf16)
        bb_sb = const.tile([128, 2], f32)
        # zero tiles for PE warm-up matmuls (no DMA dependency); a small
        # one on GpSimd unblocks N=128 warms ~600ns before the Vector
        # memset of the N=512 operand completes.
        zts = const.tile([128, 128], bf16)
        zt = const.tile([128, 512], bf16)
        nc.gpsimd.memset(zts[:], 0.0)
        nc.vector.memset(zt[:], 0.0)

        def col(i):  # first col of tile i's block
            return XB + i * TILE_COLS

        # DMA-descriptor issue on Sync costs ~0.65us per dma_start; chunk
        # sizes grow so each chunk's completion sem fires just before its
        # first consumer (tile cadence ~0.88us, measured stream rate
        # ~0.27 GB/ms under 8-core contention).  Tiles 0 and 1 get their
        # own chunks (and are processed as singles below) so the first
        # exp is gated on 393 KB instead of 525 KB.
        for a, b in [(0, 1), (1, 2), (2, 4), (4, 7), (7, 11), (11, 17), (17, 25)]:
            nc.sync.dma_start(xall_sb[:, (col(a) if a else 0) : col(b)],
                              xall_d[:, (col(a) if a else 0) : col(b)])
        nc.sync.dma_start(xall_sb[:, col(25) :], xall_d[:, col(25) :])
        nc.sync.dma_start(wf_sb[:], wf_d[:])
        nc.sync.dma_start(bb_sb[:], bb_d[:])

        def wk(k):  # rhs: w_k^T block for C-rows k*128..+128 -> [128, 512]
            return xall_sb[:, k * HID : (k + 1) * HID]

        def xs(k, i):  # lhsT: x rows k-block, spatial tile i -> [128, 128]
            return xall_sb[:, col(i) + k * 128 : col(i) + (k + 1) * 128]

        def xt(i):  # rhs: [x^T | 1] for spatial tile i -> [128, 257]
            return xall_sb[:, col(i) + 256 : col(i) + TILE_COLS]

        # x-tile columns as [128, tile, col] for the final streamed matmul
        x_tiles = xall_sb[:, XB:].rearrange("p (i t) -> p i t", t=TILE_COLS)

        def wq_h(h):
            return wf_sb[:, h * C : (h + 1) * C]

        def u_h(h, cb):  # U_h[c'-blk] = (W_v,h^T w_out_h^T) rows c'-blk
            o = 1024 + (2 * h + cb) * C
            return wf_sb[:, o : o + C]

        rsum = const.tile([128, HEADS], f32)
        gn_sb = const.tile([128, HEADS * C], bf16)
        p_sb = const.tile([128, HEADS * 2 * C], bf16)
        w_sb = const.tile([128, 2 * C], bf16)

        # ---- persistent PSUM layout (8 banks) ----
        # g[h] owns bank h ([:, 0:257] holds G_h; the full [:, 0:512]
        # is reused as a final-phase matmul target).  pk2[0/1] own banks
        # 4-5 / 6-7; their slices are reused for P and W^T in the join.
        g = [psum.tile([128, 512], f32, name=f"G{h}") for h in range(HEADS)]
        pk2 = [psum.tile([128, 1024], f32, name=f"pk2{a}") for a in range(2)]

        # ---- warm-ups: keep the PE busy (HAM at 8/8) until data lands ----
        for _ in range(5):
            nc.tensor.matmul(pk2[1][:, 0:128], zts[:], zts[:])
        for _ in range(7):
            nc.tensor.matmul(pk2[1][:, 0:HID], zts[:], zt[:])
        for _ in range(3):
            nc.tensor.matmul(pk2[1][:, 0:128], zts[:], zts[:])

        # ---- Phase 1: k^T projection + exp + G accumulation ----
        # groups: tiles 0 and 1 as singles (earlier first exp), then pairs.
        groups = [[0], [1]] + [[i, i + 1] for i in range(2, NT, 2)]
        with tc.tile_pool(name="ekp", bufs=3) as ekp:

            def emit_g(ek2, tiles, base):
                for t, i in enumerate(tiles):
                    for h in range(HEADS):
                        nc.tensor.matmul(
                            g[h][:, 0:257],
                            ek2[:, base + t * HID + h * 128 : base + t * HID + (h + 1) * 128],
                            xt(i),
                            start=(i == 0),
                            stop=(i == NT - 1),
                            skip_group_check=True,
                        )

            pending = []
            for gi, tiles in enumerate(groups):
                if gi < 2:  # singles share pk2[gi]'s first bank
                    pkt = pk2[gi][:, 0:HID]
                else:
                    pkt = pk2[gi % 2][:, 0 : HID * len(tiles)]
                for t, i in enumerate(tiles):
                    for k in range(2):
                        nc.tensor.matmul(
                            pkt[:, t * HID : (t + 1) * HID],
                            xs(k, i),
                            wk(k),
                            start=(k == 0),
                            stop=(k == 1),
                        )
                ek2 = ekp.tile([128, 1024], bf16, name="ek2")
                w_cols = HID * len(tiles)
                if gi == len(groups) - 1:
                    # split the last exps per tile so the trailing G matmuls
                    # of tile 30 can start while tile 31's exp runs
                    nc.scalar.activation(ek2[:, 0:HID], pkt[:, 0:HID], AFT.Exp)
                    nc.scalar.activation(
                        ek2[:, HID:1024], pkt[:, HID:1024], AFT.Exp
                    )
                else:
                    nc.scalar.activation(ek2[:, 0:w_cols], pkt[:], AFT.Exp)
                # software-pipeline the G matmuls one group behind so the
                # tensor engine never stalls on the exp of the same group
                pending.append((ek2, tiles))
                if len(pending) > 1:
                    e, tt = pending.pop(0)
                    emit_g(e, tt, 0)
            e, tt = pending.pop(0)
            emit_g(e, tt, 0)

            # rowsum reciprocals (DVE for accuracy), then drain+normalize
            # G into SBUF bf16, full-width, alternating Scalar/Vector.
            # (GpSimd cannot read PSUM through the NEFF path.)
            for h in range(HEADS):
                nc.vector.reciprocal(rsum[:, h : h + 1], g[h][:, 256:257])
            for h in range(HEADS):
                dst = gn_sb[:, h * C : (h + 1) * C]
                if h % 2 == 0:
                    nc.scalar.mul(dst, g[h][:, 0:C], rsum[:, h : h + 1])
                else:
                    nc.vector.tensor_scalar_mul(
                        dst, g[h][:, 0:C], rsum[:, h : h + 1]
                    )

            # ---- Phase 2a: collapse weights (P -> W^T), PSUM-aliased ----
            # Each head's P pair lives in its OWN bank (pk2[0]'s two banks
            # for h0/h1, the already-drained g0/g1 banks for h2/h3) so all
            # 8 P matmuls stream back-to-back with no copy in the chain;
            # the cb=1 matmul lands in the same bank with start=False (the
            # bank was zeroed by cb=0's start=True).  wt[cb] live in
            # pk2[1]'s two banks.
            wt = [pk2[1][:, cb * HID : cb * HID + C] for cb in range(2)]
            p_banks = [
                pk2[0][:, 0:HID],
                pk2[0][:, HID:1024],
                g[0][:, 0:HID],
                g[1][:, 0:HID],
            ]
            p_tiles = {}

            # two bridging warm-ups: keep the PE from re-throttling during
            # the recip+drain window (they write wt's banks, which the
            # first W matmul's start=True re-zeroes anyway)
            for _ in range(2):
                nc.tensor.matmul(pk2[1][:, 0:HID], zts[:], zt[:])

            def emit_p(h):
                for cb in range(2):
                    pt = p_banks[h][:, cb * C : (cb + 1) * C]
                    nc.tensor.matmul(
                        pt,
                        gn_sb[:, h * C + cb * 128 : h * C + (cb + 1) * 128],
                        wq_h(h),
                        start=(cb == 0),
                        stop=True,
                        skip_group_check=True,
                    )
                    p_tiles[(h, cb)] = pt

            def copy_p(h):
                # one [128,512] copy per head: both P tiles are adjacent in
                # the head's bank, and the per-instruction overhead (~230ns)
                # makes one wide copy cheaper than two halves
                dst = p_sb[:, h * 2 * C : (h + 1) * 2 * C]
                if h % 2 == 0:
                    nc.scalar.copy(dst, p_banks[h][:, 0:HID])
                else:
                    nc.vector.tensor_copy(dst, p_banks[h][:, 0:HID])

            def emit_w(h):
                # W^T[c-blk, o] += sum_{c'-blk} P_h[c'-blk, c-blk]^T @ U_h
                for pb in range(2):
                    for cb in range(2):
                        nc.tensor.matmul(
                            wt[cb],
                            p_sb[:, h * 2 * C + pb * C + cb * 128 : h * 2 * C + pb * C + cb * 128 + 128],
                            u_h(h, pb),
                            start=(h == 0 and pb == 0),
                            stop=(h == HEADS - 1 and pb == 1),
                            skip_group_check=True,
                        )

            for h in range(HEADS):
                emit_p(h)
            for h in range(HEADS):
                copy_p(h)
            for h in range(HEADS):
                emit_w(h)

            # quarter-width W^T drains: the first final-phase matmul needs
            # only w_sb[0:128] and w_sb[256:384], so it launches after one
            # ~340ns quarter on each engine instead of a full 470ns copy
            nc.scalar.copy(w_sb[:, 0:128], wt[0][:, 0:128])
            nc.vector.tensor_copy(w_sb[:, C : C + 128], wt[1][:, 0:128])
            nc.scalar.copy(w_sb[:, 128:C], wt[0][:, 128:C])
            nc.vector.tensor_copy(w_sb[:, C + 128 : 2 * C], wt[1][:, 128:C])

        # ---- Phase 2b: out = W @ x + b, streamed over 8 chunks of 512
        # columns; PSUM targets alias the (drained) G banks.  One output
        # DMA per chunk covers both 128-row blocks, except the last chunk
        # which is split per row-block so the final HBM write is 128 KB.
        with tc.tile_pool(name="fop", bufs=4) as fop:
            for c in range(8):
                fo = fop.tile([128, 1024], bf16, name="fo")
                for mo in range(2):
                    # cycle the G banks starting at g2 (drained earliest;
                    # g0/g1 hold P(2)/P(3) until their copies complete)
                    fp_ = g[(2 + 2 * c + mo) % 4][:, 0:512]
                    for cb in range(2):
                        nc.tensor.matmul(
                            fp_,
                            w_sb[:, cb * C + mo * 128 : cb * C + mo * 128 + 128],
                            x_tiles[:, 4 * c : 4 * c + 4, cb * 128 : (cb + 1) * 128],
                            start=(cb == 0),
                            stop=(cb == 1),
                        )
                    half = fo[:, mo * 512 : (mo + 1) * 512]
                    if c == 7:
                        # last chunk: split each bias-drain across BOTH
                        # engines so the final DMAs are gated on ~350ns of
                        # drain instead of ~750ns
                        nc.scalar.activation(
                            half[:, 0:256],
                            fp_[:, 0:256],
                            AFT.Identity,
                            bias=bb_sb[:, mo : mo + 1],
                        )
                        nc.vector.tensor_scalar_add(
                            half[:, 256:512], fp_[:, 256:512], bb_sb[:, mo : mo + 1]
                        )
                        nc.sync.dma_start(
                            out_d[:, c * 1024 + mo * 512 : c * 1024 + (mo + 1) * 512],
                            half,
                        )
                    elif mo == 0:
                        nc.scalar.activation(
                            half, fp_, AFT.Identity, bias=bb_sb[:, 0:1]
                        )
                    else:
                        nc.vector.tensor_scalar_add(half, fp_, bb_sb[:, 1:2])
                # chunk-major output layout: one contiguous 2 KiB row per
                # partition per chunk (half the DMA descriptors vs split
                # row-blocks); the host unshuffles [p, c, mo, j].
                if c < 7:
                    nc.sync.dma_start(out_d[:, c * 1024 : (c + 1) * 1024], fo[:])

    nc.compile()
    return nc


def _get_program():
    if "nc" not in _BUILD_CACHE:
        _BUILD_CACHE["nc"] = _build_program()
    return _BUILD_CACHE["nc"]


def _pack_weights(w_qkv, w_out, b_out):
    """Shared (per-core-identical) input tensors: wk header cols of the bf16
    stream, the bf16 weight stream (wq | U), and the bias pair."""
    import ml_dtypes

    bf16 = ml_dtypes.bfloat16
    w_q = np.ascontiguousarray(w_qkv[0:HID]).astype(np.float32)  # [512, 256]
    w_k = np.ascontiguousarray(w_qkv[HID : 2 * HID]).astype(np.float32)
    w_v = np.ascontiguousarray(w_qkv[2 * HID : 3 * HID]).astype(np.float32)

    def pack_rows(w):  # w [512, 256] -> [128, 4*256], block h = rows h*128:+128
        return w.reshape(HEADS, 128, C).transpose(1, 0, 2).reshape(128, HEADS * C)

    wk = np.ascontiguousarray(
        w_k.T.reshape(2, 128, HID).transpose(1, 0, 2).reshape(128, 2 * HID)
    ).astype(bf16)

    # U_h = W_v,h^T @ w_out[:, h-block]^T  [256 c', 256 o]; block (h, cb) on
    # partitions = c' within block cb
    w_outf = np.ascontiguousarray(w_out).astype(np.float32)
    u_blocks = []
    for h in range(HEADS):
        u = w_v[h * 128 : (h + 1) * 128].T @ w_outf[:, h * 128 : (h + 1) * 128].T
        u_blocks.append(u.reshape(2, 128, C).transpose(1, 0, 2).reshape(128, 2 * C))
    wf = np.concatenate([pack_rows(w_q)] + u_blocks, axis=1)
    return {
        "wk": wk,  # [128, 1024] bf16 header of the xall stream
        "wf": np.ascontiguousarray(wf.astype(bf16)),
        "bb": np.ascontiguousarray(b_out.reshape(2, 128).T).astype(np.float32),
    }


def _pack_x(xb, wk):
    """Per-batch bf16 stream: [wk | per tile i: x k-blocks (256) | x^T|1 (257)].

    xb is [256, 4096] float32."""
    import ml_dtypes

    bf16 = ml_dtypes.bfloat16
    xbh = xb.astype(bf16)
    # x-tile part: [p, i, k, j] = x[k*128+p, i*128+j]
    xs = xbh.reshape(2, 128, NT, 128).transpose(1, 2, 0, 3).reshape(128, NT, 256)
    # xT part: [p, i, c] = x[c, i*128+p]
    xt = xbh.reshape(256, NT, 128).transpose(2, 1, 0)
    ones = np.ones((128, NT, 1), dtype=bf16)
    tiles = np.concatenate([xs, xt, ones], axis=2).reshape(128, NT * TILE_COLS)
    return np.ascontiguousarray(np.concatenate([wk, tiles], axis=1))


def _ensure_ntff_hook():
    """Make trace-mode grading (BASS_TRACE=1) work even when the container's
    ``antenv`` stub lacks ``axon_hooks``: install the registry module and, if
    the axon PJRT library is present, register the ctypes NTFF profile hook."""
    import os
    import sys
    import types

    try:
        import antenv.axon_hooks  # noqa: F401
    except ImportError:
        try:
            import antenv
        except ImportError:
            return
        mod = types.ModuleType("antenv.axon_hooks")
        mod._hook = None
        mod.set_axon_ntff_profile_hook = lambda h: setattr(mod, "_hook", h)
        mod.get_axon_ntff_profile_hook = lambda: getattr(mod, "_hook", None)
        sys.modules["antenv.axon_hooks"] = mod
        antenv.axon_hooks = mod
    try:
        from antenv.axon_hooks import (
            get_axon_ntff_profile_hook,
            set_axon_ntff_profile_hook,
        )

        so = "/opt/axon/libaxon_pjrt.so"
        if get_axon_ntff_profile_hook() is None and os.path.exists(so):
            from trn_agent_boot.trn_boot import _ntff_profile_via_ctypes

            hook = _ntff_profile_via_ctypes(so)
            if hook is not None:
                set_axon_ntff_profile_hook(hook)
    except Exception:
        pass


def _make_in_maps(x, w_qkv, w_out, b_out):
    packed = _pack_weights(
        np.asarray(w_qkv, np.float32),
        np.asarray(w_out, np.float32),
        np.asarray(b_out, np.float32),
    )
    wk = packed.pop("wk")
    x = np.asarray(x, dtype=np.float32)
    return [
        {"xall": _pack_x(x[b].reshape(C, N), wk), **packed}
        for b in range(x.shape[0])
    ]


def kernel(x, w_qkv, w_out, b_out):
    from concourse.bass_utils import run_bass_kernel_spmd

    _ensure_ntff_hook()

    x = np.asarray(x, dtype=np.float32)
    B = x.shape[0]
    assert B == NCORES and x.shape[1:] == (C, 64, 64)

    nc = _get_program()
    in_maps = _make_in_maps(x, w_qkv, w_out, b_out)
    res = run_bass_kernel_spmd(nc, in_maps, core_ids=list(range(NCORES)))
    # out is chunk-major: raw[p, c*1024 + mo*512 + j] = out[mo*128+p, c*512+j]
    out = np.stack(
        [
            np.asarray(res.results[b]["out"], dtype=np.float32)
            .reshape(128, 8, 2, 512)
            .transpose(2, 0, 1, 3)
            for b in range(B)
        ],
        axis=0,
    )
    return out.reshape(B, C, 64, 64).astype(np.float32)
